# revision 11
# baseline (speedup 1.0000x reference)
"""Causal multi-head attention layer on 8 Trainium2 NeuronCores.

Sharding: tensor-parallel over heads (16 heads -> 2 per core).
Per core, for its 2 heads:
  qkv^T = W_slice^T @ x^T      t=0 token block in bf16 (precision for
                               short-history queries), t>=1 blocks via
                               fp8e4m3 DoubleRow matmuls (2 E-chunks/inst)
  S^T[k,q] = K^T_chunk^T @ Q^T (two heads packed in PE row halves via
                               tile_position; q<512 region bf16, rest fp8)
  att^T = exp(S^T/8)           (ACT; bf16 for q<512, fp8e4m3 elsewhere,
                               causal-trimmed + triangular mask on diagonal)
  out^T[dv,q] = (V|1)^T-stationary @ att^T  -> row 64 = softmax denominator
     q<512: bf16 V tiles; q>=512: fp8 DoubleRow over 256-key chunk pairs
     (above-diagonal slivers of paired chunks memset to 0)
  attout^T = out^T[0:64] * bcast(1/denom)
  partial^T[e,tok] = W_out_slice chunks @ attout^T   -> DRAM (bf16)
Host: sum partials over cores, transpose, + b_out.
"""
import os
import numpy as np
import ml_dtypes

import concourse.bacc as bacc
import concourse.bass as bass
import concourse.mybir as mybir
import concourse.tile as tile
from concourse import bass_utils

B, S, E, H = 4, 2048, 1024, 16
D = E // H            # 64
TOK = B * S           # 8192
KC = E // 128         # 8 emb chunks
TB = 512              # qkv token block
QB = 1024             # attention q block
NB = S // TB          # 4 token blocks per batch
NQB = S // QB         # 2 q blocks per batch

f32 = mybir.dt.float32
f32r = mybir.dt.float32r
bf16 = mybir.dt.bfloat16
fp8 = mybir.dt.float8e4
FT = mybir.ActivationFunctionType
DR = mybir.MatmulPerfMode.DoubleRow


def splits(lo, hi, step=512):
    """Split [lo, hi) into pieces aligned to `step` boundaries."""
    out = []
    p = lo
    while p < hi:
        q = min((p // step + 1) * step, hi)
        out.append((p, q))
        p = q
    return out


def build(repeats: int = 1):
    nc = bacc.Bacc("TRN2", target_bir_lowering=False, debug=False, num_devices=8)
    xT8 = nc.dram_tensor("xT8", [E, TOK], fp8, kind="ExternalInput")
    xT0 = nc.dram_tensor("xT0", [E, B * TB], bf16, kind="ExternalInput")
    wq8 = nc.dram_tensor("wq8", [E, 128], fp8, kind="ExternalInput")
    wk8 = nc.dram_tensor("wk8", [E, 128], fp8, kind="ExternalInput")
    wv8 = nc.dram_tensor("wv8", [E, 128], fp8, kind="ExternalInput")
    wqb = nc.dram_tensor("wqb", [E, 128], bf16, kind="ExternalInput")
    wkb = nc.dram_tensor("wkb", [E, 128], bf16, kind="ExternalInput")
    wvb = nc.dram_tensor("wvb", [E, 128], bf16, kind="ExternalInput")
    wo = nc.dram_tensor("wo", [128, E], f32r, kind="ExternalInput")
    bq = nc.dram_tensor("bq", [128, 1], f32, kind="ExternalInput")
    bk = nc.dram_tensor("bk", [128, 1], f32, kind="ExternalInput")
    bv = nc.dram_tensor("bv", [128, 1], f32, kind="ExternalInput")
    tri = nc.dram_tensor("tri", [128, 128], bf16, kind="ExternalInput")
    idd = nc.dram_tensor("idd", [128, 128], bf16, kind="ExternalInput")
    outp = nc.dram_tensor("outp", [E, TOK], bf16, kind="ExternalOutput")

    def pair3(ap2d, blk, k=2):
        """[p, 2*blk] slice -> [p, 2, blk] AP for DoubleRow k-tile pairs."""
        return ap2d.rearrange("p (k m) -> p k m", k=k)

    with tile.TileContext(nc) as tc:
        with (
            tc.tile_pool(name="wp", bufs=1) as wp,
            tc.tile_pool(name="xp", bufs=2) as xp,
            tc.tile_pool(name="qk", bufs=2) as qk,
            tc.tile_pool(name="vn", bufs=1) as vnp,
            tc.tile_pool(name="at", bufs=2) as atp,
            tc.tile_pool(name="ao", bufs=2) as aop,
            tc.tile_pool(name="ms", bufs=1) as ms,
            tc.tile_pool(name="op", bufs=3) as op,
            tc.tile_pool(name="psA", bufs=2, space="PSUM") as psA,
            tc.tile_pool(name="psS", bufs=1, space="PSUM") as psS,
            tc.tile_pool(name="psO", bufs=1, space="PSUM") as psO,
        ):
            # --- constants / weights (loaded once) ---
            # first QKV token block's x slice loads FIRST so the PE can
            # start as soon as the weights land; remaining weights follow.
            xt00 = []
            for hf in range(2):
                x1 = xp.tile([128, KC * TB // 2], bf16, tag=f"xb{hf}",
                             name=f"xb_pre0_{hf}")
                nc.sync.dma_start(
                    x1[:].rearrange("p (c m) -> p c m", c=KC // 2),
                    xT0.ap()[hf * (E // 2):(hf + 1) * (E // 2), 0:TB].rearrange(
                        "(c p) m -> p c m", p=128),
                )
                xt00.append(x1)
            wqb_sb = wp.tile([128, E], bf16)
            wkb_sb = wp.tile([128, E], bf16)
            wvb_sb = wp.tile([128, E], bf16)
            wq8_sb = wp.tile([128, E], fp8)
            wk8_sb = wp.tile([128, E], fp8)
            wv8_sb = wp.tile([128, E], fp8)
            bq_sb = wp.tile([128, 1], f32)
            bk_sb = wp.tile([128, 1], f32)
            bv_sb = wp.tile([128, 1], f32)
            wo_sb = wp.tile([128, E], f32r)
            for hf in range(2):
                nc.sync.dma_start(
                    wqb_sb[:, hf * (E // 2):(hf + 1) * (E // 2)].rearrange(
                        "p (c m) -> p c m", c=KC // 2),
                    wqb.ap()[hf * (E // 2):(hf + 1) * (E // 2), :].rearrange(
                        "(c p) m -> p c m", p=128),
                )
            nc.sync.dma_start(bq_sb[:], bq.ap())
            for wsb_, wdr_ in (
                (wkb_sb, wkb), (wvb_sb, wvb),
                (wq8_sb, wq8), (wk8_sb, wk8), (wv8_sb, wv8),
            ):
                nc.sync.dma_start(
                    wsb_[:].rearrange("p (c m) -> p c m", c=KC),
                    wdr_.ap().rearrange("(c p) m -> p c m", p=128),
                )
            nc.sync.dma_start(wo_sb[:], wo.ap())
            nc.sync.dma_start(bk_sb[:], bk.ap())
            nc.sync.dma_start(bv_sb[:], bv.ap())
            tri_sb = wp.tile([128, 128], bf16)
            nc.sync.dma_start(tri_sb[:], tri.ap())
            id_sb = wp.tile([128, 128], bf16)
            nc.sync.dma_start(id_sb[:], idd.ap())
            # preload ACT exp table set during the prologue
            warm = wp.tile([1, 1], f32)
            nc.vector.memset(warm[:], 0.0)
            nc.scalar.activation(warm[:], warm[:], FT.Exp, scale=1.0)
            # persistent V tiles. bf16 vns (chunks 0..3) serve the q<512
            # AV path; fp8 vn8 pair-tiles serve DoubleRow AV: per pair p
            # the layout is [h0c0|1 @0, h0c1|1 @80, h1c0|1 @160, h1c1|1 @240]
            # (ones at 64/144/224/304), so head h's stationary is the
            # [80-stride, 2][1, 65] AP from base 160h.
            vns = []
            for i in range(4):
                vn = vnp.tile([128, 130], bf16, tag=f"vn{i}", name=f"vn{i}")
                nc.vector.memset(vn[:, 64:65], 1.0)
                nc.vector.memset(vn[:, 129:130], 1.0)
                vns.append(vn)
            vn8s = []
            for p in range(8):
                vn8 = vnp.tile([128, 320], fp8, tag=f"vn8_{p}", name=f"vn8_{p}")
                for col in (64, 144, 224, 304):
                    nc.vector.memset(vn8[:, col:col + 1], 1.0)
                vn8s.append(vn8)

            def alloc_qkv(b):
                return (
                    qk.tile([128, TB], bf16, tag="qTb", name=f"qTb{b}"),
                    qk.tile([128, TB], bf16, tag="kTb", name=f"kTb{b}"),
                    qk.tile([128, S], fp8, tag="qT8", name=f"qT8{b}"),
                    qk.tile([128, S], fp8, tag="kT8", name=f"kT8{b}"),
                    qk.tile([128, S], bf16, tag="vT", name=f"vT{b}"),
                )

            def qkv_dma0(b, rep):
                """bf16 x for token block t=0 of batch b."""
                if b == 0 and rep == 0:
                    return xt00
                xth = []
                for hf in range(2):
                    x1 = xp.tile([128, KC * TB // 2], bf16, tag=f"xb{hf}",
                                 name=f"xb{rep}_{b}_{hf}")
                    nc.sync.dma_start(
                        x1[:].rearrange("p (c m) -> p c m", c=KC // 2),
                        xT0.ap()[hf * (E // 2):(hf + 1) * (E // 2),
                                 b * TB:(b + 1) * TB].rearrange(
                            "(c p) m -> p c m", p=128),
                    )
                    xth.append(x1)
                return xth

            def qkv_dma8(b, t, rep):
                """fp8 x for token block t (>=1) of batch b."""
                tok0 = b * S + t * TB
                xth = []
                for hf in range(2):
                    x1 = xp.tile([128, KC * TB // 2], fp8, tag=f"x8{hf}",
                                 name=f"x8{rep}_{b}_{t}_{hf}")
                    nc.sync.dma_start(
                        x1[:].rearrange("p (c m) -> p c m", c=KC // 2),
                        xT8.ap()[hf * (E // 2):(hf + 1) * (E // 2),
                                 tok0:tok0 + TB].rearrange(
                            "(c p) m -> p c m", p=128),
                    )
                    xth.append(x1)
                return xth

            def qkv_t0(b, tiles, xth, gi, rep):
                """bf16 QKV for t=0; writes bf16 q/k/v (+ fp8 k copy)."""
                qTb, kTb, qT8, kT8, vT = tiles
                wsb, bsb = ((wqb_sb, bq_sb), (wkb_sb, bk_sb),
                            (wvb_sb, bv_sb))[gi]
                ps = psA.tile([128, TB], f32, tag="mm512",
                              name=f"psq0_{rep}_{b}_{gi}")
                for kc in range(KC):
                    xsrc = xth[kc // (KC // 2)]
                    nc.tensor.matmul(
                        ps[:],
                        wsb[:, kc * 128:(kc + 1) * 128],
                        xsrc[:, (kc % (KC // 2)) * TB:
                             (kc % (KC // 2) + 1) * TB],
                        start=(kc == 0), stop=(kc == KC - 1),
                    )
                if gi == 0:
                    nc.vector.tensor_scalar_add(qTb[:], ps[:], bsb[:])
                elif gi == 1:
                    nc.vector.tensor_scalar_add(kTb[:], ps[:], bsb[:])
                    nc.vector.tensor_scalar_add(kT8[:, 0:TB], ps[:], bsb[:])
                else:
                    nc.vector.tensor_scalar_add(vT[:, 0:TB], ps[:], bsb[:])

            def qkv_dr(b, t, tiles, xth, gi, rep):
                """fp8 DoubleRow QKV for t>=1; dst fp8 q/k, bf16 v."""
                qTb, kTb, qT8, kT8, vT = tiles
                w8, bsb, dst = (
                    (wq8_sb, bq_sb, qT8), (wk8_sb, bk_sb, kT8),
                    (wv8_sb, bv_sb, vT),
                )[gi]
                ps = psA.tile([128, TB], f32, tag="mm512",
                              name=f"psq8_{rep}_{b}_{t}_{gi}")
                for pi in range(KC // 2):
                    xsrc = xth[pi // 2]
                    lo = (pi % 2) * 2 * TB
                    nc.tensor.matmul(
                        ps[:],
                        pair3(w8[:, pi * 256:(pi + 1) * 256], 128),
                        pair3(xsrc[:, lo:lo + 2 * TB], TB),
                        start=(pi == 0), stop=(pi == KC // 2 - 1),
                        perf_mode=DR,
                    )
                nc.vector.tensor_scalar_add(
                    dst[:, t * TB:(t + 1) * TB], ps[:], bsb[:]
                )

            def vnat(b, tiles, rep, lo=0, hi=S // 128):
                vT = tiles[4]
                for i in range(lo, hi):
                    pst = psA.tile([128, 128], bf16, tag="mm512",
                                   name=f"pst{rep}_{b}_{i}")
                    nc.tensor.transpose(
                        pst[:], vT[:, i * 128:(i + 1) * 128], id_sb[:]
                    )
                    src = pst[:, 0:64]
                    src3 = bass.AP(src.tensor, src.offset,
                                   [src.ap[0], [64, 2], [1, 64]])
                    if i < 4:
                        # bf16 copy into both 64-col head groups of vns[i]
                        dst = vns[i][:, 0:64]
                        dst3 = bass.AP(dst.tensor, dst.offset,
                                       [dst.ap[0], [65, 2], [1, 64]])
                        nc.vector.tensor_copy(dst3, src3)
                    # fp8 pair copy: heads at 160-stride, chunk parity at 80
                    d = vn8s[i // 2][:, 80 * (i % 2):80 * (i % 2) + 64]
                    d3 = bass.AP(d.tensor, d.offset,
                                 [d.ap[0], [160, 2], [1, 64]])
                    nc.vector.tensor_copy(d3, src3)

            def att8_zero(qb, att8, h, qbb):
                """Zero above-diagonal slivers of paired chunks so the
                DoubleRow AV can read full 512-wide windows."""
                j = 2 * qb + qbb
                q0 = qb * QB
                qa0 = j * 512
                hb = h * 16 * QB
                for kcd in range(4 * j, 4 * j + 4):
                    kst = kcd * 128
                    if kst > qa0:
                        nc.vector.memset(
                            att8[:, hb + kcd * QB + (qa0 - q0):
                                 hb + kcd * QB + (kst - q0)], 0.0)

            def scores(b, qb, tiles, attbf, att8, rep, fill=()):
                qTb, kTb, qT8, kT8, vT = tiles
                q0 = qb * QB
                nkc = (q0 + QB) // 128
                fill = list(fill)
                nf = len(fill)
                fired = 0
                pi = 0
                for kc in range(nkc):
                    kst = kc * 128
                    r0 = max(0, kst - q0)
                    # two 512-col half-windows, double-buffered PSUM (sA/sB)
                    for w in range(2):
                        wlo, whi = w * 512, (w + 1) * 512
                        h0 = max(r0, wlo)
                        if h0 >= whi:
                            continue
                        ps_s = psS.tile([128, 2 * 512], f32,
                                        tag=f"s{pi % 2}",
                                        name=f"pss{rep}_{b}_{qb}_{kc}_{w}")
                        pi += 1
                        for h in range(2):
                            hs = slice(h * 64, (h + 1) * 64)
                            hq = h * 512
                            if qb == 0 and kc < 4 and w == 0:
                                nc.tensor.matmul(
                                    ps_s[:, hq + h0 - wlo:hq + 512],
                                    kTb[hs, kst:kst + 128],
                                    qTb[hs, h0:whi],
                                    start=True, stop=True,
                                    tile_position=(h * 64, 0),
                                )
                            else:
                                nc.tensor.matmul(
                                    ps_s[:, hq + h0 - wlo:hq + 512],
                                    kT8[hs, kst:kst + 128],
                                    qT8[hs, q0 + h0:q0 + whi],
                                    start=True, stop=True,
                                    tile_position=(h * 64, 0),
                                )
                        # per-head exp for this half-window
                        n = whi - h0
                        for h in range(2):
                            if qb == 0 and kc < 4 and w == 0:
                                nc.scalar.activation(
                                    attbf[:, h * 4 * 512 + kc * 512 + h0:
                                          h * 4 * 512 + kc * 512 + h0 + n],
                                    ps_s[:, h * 512 + h0 - wlo:
                                         h * 512 + h0 - wlo + n],
                                    FT.Exp, scale=0.125)
                            else:
                                nc.scalar.activation(
                                    att8[:, h * 16 * QB + kc * QB + h0:
                                         h * 16 * QB + kc * QB + h0 + n],
                                    ps_s[:, h * 512 + h0 - wlo:
                                         h * 512 + h0 - wlo + n],
                                    FT.Exp, scale=0.125)
                        # triangular mask on the diagonal block
                        if r0 == h0 and kst >= q0:
                            for h in range(2):
                                if qb == 0 and kc < 4:
                                    blk = attbf[:, h * 4 * 512 + kc * 512
                                                + r0:
                                                h * 4 * 512 + kc * 512
                                                + r0 + 128]
                                else:
                                    blk = att8[:, h * 16 * QB + kc * QB + r0:
                                               h * 16 * QB + kc * QB
                                               + r0 + 128]
                                nc.vector.tensor_tensor(
                                    blk, blk, tri_sb[:],
                                    op=mybir.AluOpType.mult)
                    # interleave PE fill work (next batch QKV, outproj) to
                    # cover the ACT exp-throughput deficit
                    want = (kc + 1) * nf // nkc
                    while fired < want:
                        fill[fired]()
                        fired += 1
                while fired < nf:
                    fill[fired]()
                    fired += 1

            def normalize(b, h, qa0, ps_o, aos, rep):
                rec = ms.tile([1, 512], f32, tag=f"rec{h}",
                              name=f"rec{rep}_{b}_{qa0}_{h}")
                nc.vector.reciprocal(rec[:], ps_o[64:65, :])
                bc = ms.tile([64, 512], f32, tag=f"bc{h}",
                             name=f"bc{rep}_{b}_{qa0}_{h}")
                nc.gpsimd.partition_broadcast(bc[:], rec[:])
                nc.vector.tensor_tensor(
                    aos[h * 64:(h + 1) * 64, qa0:qa0 + 512],
                    ps_o[0:64, :], bc[:],
                    op=mybir.AluOpType.mult,
                )

            def attv_bf(b, h, attbf, aos, rep):
                """q in [0, 512): bf16 AV over chunks 0..3 with trimming."""
                ps_o = psO.tile([65, 512], f32, tag=f"o{h}",
                                name=f"pso{rep}_{b}_0_{h}")
                for kc in range(4):
                    lo = kc * 128
                    vn = vns[kc]
                    nc.tensor.matmul(
                        ps_o[:, lo:512],
                        vn[:, h * 65:(h + 1) * 65],
                        attbf[:, h * 4 * 512 + kc * 512 + lo:
                              h * 4 * 512 + (kc + 1) * 512],
                        start=(kc == 0), stop=(kc == 3),
                    )
                normalize(b, h, 0, ps_o, aos, rep)

            def attv_dr(b, qb, qbb, h, att8, aos, rep):
                """q block j=2qb+qbb >= 1: fp8 DoubleRow AV, 256-key pairs."""
                j = 2 * qb + qbb
                qoff = qbb * 512
                qa0 = j * 512
                npairs = 2 * j + 2
                ps_o = psO.tile([65, 512], f32, tag=f"o{h}",
                                name=f"pso{rep}_{b}_{j}_{h}")
                for p in range(npairs):
                    v0 = vn8s[p][:, 160 * h:160 * h + 65]
                    st = bass.AP(v0.tensor, v0.offset,
                                 [v0.ap[0], [80, 2], [1, 65]])
                    a0 = att8[:, h * 16 * QB + (2 * p) * QB + qoff:
                              h * 16 * QB + (2 * p) * QB + qoff + 512]
                    mv = bass.AP(a0.tensor, a0.offset,
                                 [a0.ap[0], [QB, 2], [1, 512]])
                    nc.tensor.matmul(
                        ps_o[:], st, mv,
                        start=(p == 0), stop=(p == npairs - 1),
                        perf_mode=DR,
                    )
                normalize(b, h, qa0, ps_o, aos, rep)

            def outproj_ec(b, half, ec, aos, rep, eng="alt"):
                t0b = b * S
                po = op.tile([128, S // 2], bf16, tag="po",
                             name=f"po{rep}_{b}_{ec}_{half}")
                for tt in range(NB // 2):
                    t = half * (NB // 2) + tt
                    ps_p = psA.tile([128, TB], f32, tag="mm512",
                                    name=f"psp{rep}_{b}_{ec}_{t}")
                    nc.tensor.matmul(
                        ps_p[:],
                        wo_sb[:, ec * 128:(ec + 1) * 128],
                        aos[:, t * TB:(t + 1) * TB],
                        start=True, stop=True,
                    )
                    # copy engine: ScalarE only when not competing with
                    # the scores-loop exp FIFO
                    if eng == "alt" and (ec * 2 + tt) % 4 == 3:
                        nc.scalar.copy(
                            po[:, tt * TB:(tt + 1) * TB], ps_p[:]
                        )
                    else:
                        nc.vector.tensor_copy(
                            po[:, tt * TB:(tt + 1) * TB], ps_p[:]
                        )
                nc.sync.dma_start(
                    outp.ap()[ec * 128:(ec + 1) * 128,
                              t0b + half * (S // 2):
                              t0b + (half + 1) * (S // 2)],
                    po[:],
                )

            def outproj_half(b, half, aos, rep):
                for ec in range(KC):
                    outproj_ec(b, half, ec, aos, rep)

            for rep in range(repeats):
                # prologue: batch-0 token blocks 0,1 serially; the rest
                # becomes fill work inside the first scores loop
                tiles = alloc_qkv(0)
                xth = qkv_dma0(0, rep)
                for gi in range(3):
                    qkv_t0(0, tiles, xth, gi, rep)
                xth = qkv_dma8(0, 1, rep)
                for gi in range(3):
                    qkv_dr(0, 1, tiles, xth, gi, rep)
                vnat(0, tiles, rep, 0, 8)
                pro_fill = []
                for t in (2, 3):
                    xth = qkv_dma8(0, t, rep)
                    for gi in range(3):
                        pro_fill.append(
                            (lambda t=t, xth=xth, gi=gi, tl=tiles:
                             qkv_dr(0, t, tl, xth, gi, rep))
                        )
                pro_fill.append(
                    (lambda tl=tiles: vnat(0, tl, rep, 8, S // 128))
                )
                prev = None  # (b, aos) with half-1 outproj still pending
                for b in range(B):
                    nxt = b + 1 if b + 1 < B else None
                    tiles_next = alloc_qkv(nxt) if nxt is not None else None
                    aos = aop.tile([128, S], f32r, tag="ao", name=f"ao{rep}_{b}")
                    attbf = atp.tile([128, 2 * 4 * 512], bf16, tag="attbf",
                                     name=f"attbf{rep}_{b}")
                    for qb in range(NQB):
                        att8 = atp.tile([128, 2 * 16 * QB], fp8, tag="att8",
                                        name=f"att8{rep}_{b}_{qb}")
                        for h in range(2):
                            for qbb in range(2):
                                if 2 * qb + qbb >= 1:
                                    att8_zero(qb, att8, h, qbb)
                        fill = []
                        if b == 0 and qb == 0:
                            fill.extend(pro_fill)
                        if nxt is not None:
                            if qb == 0:
                                xth0 = qkv_dma0(nxt, rep)
                                for gi in range(3):
                                    fill.append(
                                        (lambda gi=gi, xth=xth0:
                                         qkv_t0(nxt, tiles_next, xth,
                                                gi, rep))
                                    )
                            else:
                                for t in (1, 2, 3):
                                    xth = qkv_dma8(nxt, t, rep)
                                    for gi in range(3):
                                        fill.append(
                                            (lambda t=t, xth=xth, gi=gi:
                                             qkv_dr(nxt, t, tiles_next,
                                                    xth, gi, rep))
                                        )
                        if qb == 0 and prev is not None:
                            pb, paos = prev
                            for ec in range(KC):
                                fill.append(
                                    (lambda ec=ec, pb=pb, paos=paos:
                                     outproj_ec(pb, 1, ec, paos, rep,
                                                eng="alt"))
                                )
                            prev = None
                        if qb == 1:
                            for ec in range(KC):
                                fill.append(
                                    (lambda ec=ec: outproj_ec(b, 0, ec,
                                                              aos, rep,
                                                              eng="alt"))
                                )
                        scores(b, qb, tiles, attbf, att8, rep, fill)
                        for qbb in range(2):
                            j = 2 * qb + qbb
                            for h in range(2):
                                if j == 0:
                                    attv_bf(b, h, attbf, aos, rep)
                                else:
                                    attv_dr(b, qb, qbb, h, att8, aos, rep)
                    if nxt is not None:
                        vnat(nxt, tiles_next, rep)
                    prev = (b, aos)
                    tiles = tiles_next
                pb, paos = prev
                outproj_half(pb, 1, paos, rep)
    nc.compile()
    return nc


_CACHE = {}


def _get_nc(repeats=1):
    if repeats not in _CACHE:
        _CACHE[repeats] = build(repeats)
    return _CACHE[repeats]


def make_in_maps(x, W_qkv, b_qkv, W_out, b_out):
    x = np.asarray(x, dtype=np.float32)
    W_qkv = np.asarray(W_qkv, dtype=np.float32)
    b_qkv = np.asarray(b_qkv, dtype=np.float32)
    W_out = np.asarray(W_out, dtype=np.float32)
    xT = np.ascontiguousarray(x.reshape(TOK, E).T)
    xT8 = xT.astype(ml_dtypes.float8_e4m3)
    xT0 = np.ascontiguousarray(
        np.concatenate([xT[:, b * S:b * S + TB] for b in range(B)], axis=1)
    ).astype(ml_dtypes.bfloat16)
    trim = np.ascontiguousarray(
        np.triu(np.ones((128, 128), dtype=np.float32))
    ).astype(ml_dtypes.bfloat16)
    in_maps = []
    for c in range(8):
        cs = slice(c * 128, (c + 1) * 128)
        wq_f = np.ascontiguousarray(W_qkv[:, c * 128:(c + 1) * 128])
        wk_f = np.ascontiguousarray(W_qkv[:, E + c * 128:E + (c + 1) * 128])
        wv_f = np.ascontiguousarray(
            W_qkv[:, 2 * E + c * 128:2 * E + (c + 1) * 128])
        in_maps.append({
            "xT8": xT8,
            "xT0": xT0,
            "wq8": wq_f.astype(ml_dtypes.float8_e4m3),
            "wk8": wk_f.astype(ml_dtypes.float8_e4m3),
            "wv8": wv_f.astype(ml_dtypes.float8_e4m3),
            "wqb": wq_f.astype(ml_dtypes.bfloat16),
            "wkb": wk_f.astype(ml_dtypes.bfloat16),
            "wvb": wv_f.astype(ml_dtypes.bfloat16),
            "wo": np.ascontiguousarray(W_out[cs, :]),
            "bq": np.ascontiguousarray(b_qkv[c * 128:(c + 1) * 128, None]),
            "bk": np.ascontiguousarray(b_qkv[E + c * 128:E + (c + 1) * 128, None]),
            "bv": np.ascontiguousarray(
                b_qkv[2 * E + c * 128:2 * E + (c + 1) * 128, None]),
            "tri": trim,
            "idd": np.eye(128, dtype=np.float32).astype(ml_dtypes.bfloat16),
        })
    return in_maps


def gather(results, b_out):
    total = np.zeros((E, TOK), dtype=np.float64)
    for c in range(8):
        total += results[c]["outp"].astype(np.float64)
    out = total.T.astype(np.float32) + np.asarray(b_out, dtype=np.float32)
    return np.ascontiguousarray(out.reshape(B, S, E)).astype(np.float32)


def kernel(x, W_qkv, b_qkv, W_out, b_out):
    nc = _get_nc(1)
    in_maps = make_in_maps(x, W_qkv, b_qkv, W_out, b_out)
    res = bass_utils.run_bass_kernel_spmd(nc, in_maps, core_ids=list(range(8)))
    return gather(res.results, b_out)


# revision 12
# speedup vs baseline: 1.2173x; 1.2173x over previous
"""Phase-0 reference kernel (baseline + bf16 x/W_qkv/outp) for A/B timing."""
import numpy as np
import ml_dtypes

import concourse.bacc as bacc
import concourse.bass as bass
import concourse.mybir as mybir
import concourse.tile as tile
from concourse import bass_utils

B, S, E, H = 4, 2048, 1024, 16
D = E // H
TOK = B * S
KC = E // 128
TB = 512
QB = 1024
NB = S // TB
NQB = S // QB

f32 = mybir.dt.float32
f32r = mybir.dt.float32r
bf16 = mybir.dt.bfloat16
FT = mybir.ActivationFunctionType


def splits(lo, hi, step=512):
    out = []
    p = lo
    while p < hi:
        q = min((p // step + 1) * step, hi)
        out.append((p, q))
        p = q
    return out


def build(repeats: int = 1):
    nc = bacc.Bacc("TRN2", target_bir_lowering=False, debug=False, num_devices=8)
    xT = nc.dram_tensor("xT", [E, TOK], bf16, kind="ExternalInput")
    wq = nc.dram_tensor("wq", [E, 128], bf16, kind="ExternalInput")
    wk = nc.dram_tensor("wk", [E, 128], bf16, kind="ExternalInput")
    wv = nc.dram_tensor("wv", [E, 128], bf16, kind="ExternalInput")
    wo = nc.dram_tensor("wo", [128, E], f32r, kind="ExternalInput")
    bq = nc.dram_tensor("bq", [128, 1], f32, kind="ExternalInput")
    bk = nc.dram_tensor("bk", [128, 1], f32, kind="ExternalInput")
    bv = nc.dram_tensor("bv", [128, 1], f32, kind="ExternalInput")
    tri = nc.dram_tensor("tri", [128, 128], bf16, kind="ExternalInput")
    idd = nc.dram_tensor("idd", [128, 128], bf16, kind="ExternalInput")
    outp = nc.dram_tensor("outp", [E, TOK], bf16, kind="ExternalOutput")

    with tile.TileContext(nc) as tc:
        with (
            tc.tile_pool(name="wp", bufs=1) as wp,
            tc.tile_pool(name="xp", bufs=2) as xp,
            tc.tile_pool(name="qk", bufs=2) as qk,
            tc.tile_pool(name="vn", bufs=1) as vnp,
            tc.tile_pool(name="at", bufs=1) as atp,
            tc.tile_pool(name="ao", bufs=2) as aop,
            tc.tile_pool(name="ms", bufs=1) as ms,
            tc.tile_pool(name="op", bufs=3) as op,
            tc.tile_pool(name="psA", bufs=2, space="PSUM") as psA,
            tc.tile_pool(name="psS", bufs=1, space="PSUM") as psS,
            tc.tile_pool(name="psO", bufs=1, space="PSUM") as psO,
        ):
            xt00 = []
            for hf in range(2):
                x1 = xp.tile([128, KC * TB // 2], bf16, tag=f"xt{hf}",
                             name=f"xt_pre0_{hf}")
                nc.sync.dma_start(
                    x1[:].rearrange("p (c m) -> p c m", c=KC // 2),
                    xT.ap()[hf * (E // 2):(hf + 1) * (E // 2), 0:TB].rearrange(
                        "(c p) m -> p c m", p=128),
                )
                xt00.append(x1)
            wq_sb = wp.tile([128, E], bf16)
            wk_sb = wp.tile([128, E], bf16)
            wv_sb = wp.tile([128, E], bf16)
            wo_sb = wp.tile([128, E], f32r)
            bq_sb = wp.tile([128, 1], f32)
            bk_sb = wp.tile([128, 1], f32)
            bv_sb = wp.tile([128, 1], f32)
            for hf in range(2):
                nc.sync.dma_start(
                    wq_sb[:, hf * (E // 2):(hf + 1) * (E // 2)].rearrange(
                        "p (c m) -> p c m", c=KC // 2),
                    wq.ap()[hf * (E // 2):(hf + 1) * (E // 2), :].rearrange(
                        "(c p) m -> p c m", p=128),
                )
            nc.sync.dma_start(bq_sb[:], bq.ap())
            for wsb_, wdr_ in ((wk_sb, wk), (wv_sb, wv)):
                nc.sync.dma_start(
                    wsb_[:].rearrange("p (c m) -> p c m", c=KC),
                    wdr_.ap().rearrange("(c p) m -> p c m", p=128),
                )
            nc.sync.dma_start(wo_sb[:], wo.ap())
            nc.sync.dma_start(bk_sb[:], bk.ap())
            nc.sync.dma_start(bv_sb[:], bv.ap())
            tri_sb = wp.tile([128, 128], bf16)
            nc.sync.dma_start(tri_sb[:], tri.ap())
            id_sb = wp.tile([128, 128], bf16)
            nc.sync.dma_start(id_sb[:], idd.ap())
            warm = wp.tile([1, 1], f32)
            nc.vector.memset(warm[:], 0.0)
            nc.scalar.activation(warm[:], warm[:], FT.Exp, scale=1.0)
            vns = []
            for i in range(S // 128):
                vn = vnp.tile([128, 130], bf16, tag=f"vn{i}", name=f"vn{i}")
                nc.vector.memset(vn[:, 64:65], 1.0)
                nc.vector.memset(vn[:, 129:130], 1.0)
                vns.append(vn)

            def alloc_qkv(b):
                return (
                    qk.tile([128, S], f32r, tag="qT", name=f"qT{b}"),
                    qk.tile([128, S], f32r, tag="kT", name=f"kT{b}"),
                    qk.tile([128, S], bf16, tag="vT", name=f"vT{b}"),
                )

            def qkv_dma(b, t, rep):
                tok0 = b * S + t * TB
                xth = []
                for hf in range(2):
                    x1 = xp.tile([128, KC * TB // 2], bf16, tag=f"xt{hf}",
                                 name=f"xt{rep}_{b}_{t}_{hf}")
                    nc.sync.dma_start(
                        x1[:].rearrange("p (c m) -> p c m", c=KC // 2),
                        xT.ap()[hf * (E // 2):(hf + 1) * (E // 2),
                                tok0:tok0 + TB].rearrange(
                            "(c p) m -> p c m", p=128),
                    )
                    xth.append(x1)
                return xth

            def qkv_group(b, t, tiles, xth, gi, rep):
                qT, kT, vT = tiles
                wsb, bsb, dst = (
                    (wq_sb, bq_sb, qT), (wk_sb, bk_sb, kT),
                    (wv_sb, bv_sb, vT),
                )[gi]
                ps = psA.tile([128, TB], f32, tag="mm512",
                              name=f"psqkv{rep}_{b}_{t}_{gi}")
                for kc in range(KC):
                    xsrc = xth[kc // (KC // 2)]
                    nc.tensor.matmul(
                        ps[:],
                        wsb[:, kc * 128:(kc + 1) * 128],
                        xsrc[:, (kc % (KC // 2)) * TB:
                             (kc % (KC // 2) + 1) * TB],
                        start=(kc == 0), stop=(kc == KC - 1),
                    )
                nc.vector.tensor_scalar_add(
                    dst[:, t * TB:(t + 1) * TB], ps[:], bsb[:]
                )

            def vnat(b, tiles, rep, lo=0, hi=S // 128):
                vT = tiles[2]
                for i in range(lo, hi):
                    vn = vns[i]
                    pst = psA.tile([128, 128], bf16, tag="mm512",
                                   name=f"pst{rep}_{b}_{i}")
                    nc.tensor.transpose(
                        pst[:], vT[:, i * 128:(i + 1) * 128], id_sb[:]
                    )
                    dst = vn[:, 0:64]
                    dst3 = bass.AP(dst.tensor, dst.offset,
                                   [dst.ap[0], [65, 2], [1, 64]])
                    src = pst[:, 0:64]
                    src3 = bass.AP(src.tensor, src.offset,
                                   [src.ap[0], [64, 2], [1, 64]])
                    nc.vector.tensor_copy(dst3, src3)

            def scores(b, qb, tiles, att, rep, fill=()):
                qT, kT, vT = tiles
                q0 = qb * QB
                nkc = (q0 + QB) // 128
                fill = list(fill)
                nf = len(fill)
                fired = 0
                pss = {}
                for kc in range(nkc):
                    kst = kc * 128
                    r0 = max(0, kst - q0)
                    for h in range(2):
                        ps_s = psS.tile([128, QB], f32, tag=f"s{h}",
                                        name=f"pss{rep}_{b}_{qb}_{kc}_{h}")
                        hs = slice(h * 64, (h + 1) * 64)
                        for (p0, p1) in splits(r0, QB):
                            nc.tensor.matmul(
                                ps_s[:, p0:p1],
                                kT[hs, kst:kst + 128],
                                qT[hs, q0 + p0:q0 + p1],
                                start=True, stop=True,
                                tile_position=(h * 64, 0),
                            )
                        pss[(kc, h)] = ps_s
                    for h in range(2):
                        ps_s = pss[(kc, h)]
                        nc.scalar.activation(
                            att[h][:, kc * QB + r0:(kc + 1) * QB],
                            ps_s[:, r0:QB],
                            FT.Exp, scale=0.125,
                        )
                        if kst >= q0:
                            blk = att[h][:, kc * QB + r0:kc * QB + r0 + 128]
                            nc.vector.tensor_tensor(
                                blk, blk, tri_sb[:],
                                op=mybir.AluOpType.mult,
                            )
                    want = (kc + 1) * nf // nkc
                    while fired < want:
                        fill[fired]()
                        fired += 1
                while fired < nf:
                    fill[fired]()
                    fired += 1

            def attv_qbb(b, qb, qbb, h, att, aos, rep):
                q0 = qb * QB
                qa0 = q0 + qbb * 512
                nkc_q = (qa0 + 512) // 128
                ps_o = psO.tile([65, 512], f32, tag=f"o{h}",
                                name=f"pso{rep}_{b}_{qb}_{qbb}_{h}")
                for kc in range(nkc_q):
                    kst = kc * 128
                    lo = max(qa0, kst) - qa0
                    vn = vns[kc]
                    nc.tensor.matmul(
                        ps_o[:, lo:512],
                        vn[:, h * 65:(h + 1) * 65],
                        att[h][:, kc * QB + qbb * 512 + lo:
                               kc * QB + (qbb + 1) * 512],
                        start=(kc == 0), stop=(kc == nkc_q - 1),
                    )
                rec = ms.tile([1, 512], f32, tag=f"rec{h}",
                              name=f"rec{rep}_{b}_{qb}_{qbb}_{h}")
                nc.vector.reciprocal(rec[:], ps_o[64:65, :])
                bc = ms.tile([64, 512], f32, tag=f"bc{h}",
                             name=f"bc{rep}_{b}_{qb}_{qbb}_{h}")
                nc.gpsimd.partition_broadcast(bc[:], rec[:])
                nc.vector.tensor_tensor(
                    aos[h * 64:(h + 1) * 64, qa0:qa0 + 512],
                    ps_o[0:64, :], bc[:],
                    op=mybir.AluOpType.mult,
                )

            def outproj_ec(b, half, ec, aos, rep, eng="alt"):
                t0b = b * S
                po = op.tile([128, S // 2], bf16, tag="po",
                             name=f"po{rep}_{b}_{ec}_{half}")
                for tt in range(NB // 2):
                    t = half * (NB // 2) + tt
                    ps_p = psA.tile([128, TB], f32, tag="mm512",
                                    name=f"psp{rep}_{b}_{ec}_{t}")
                    nc.tensor.matmul(
                        ps_p[:],
                        wo_sb[:, ec * 128:(ec + 1) * 128],
                        aos[:, t * TB:(t + 1) * TB],
                        start=True, stop=True,
                    )
                    if eng == "alt" and (ec * 2 + tt) % 4 == 3:
                        nc.scalar.copy(
                            po[:, tt * TB:(tt + 1) * TB], ps_p[:]
                        )
                    else:
                        nc.vector.tensor_copy(
                            po[:, tt * TB:(tt + 1) * TB], ps_p[:]
                        )
                nc.sync.dma_start(
                    outp.ap()[ec * 128:(ec + 1) * 128,
                              t0b + half * (S // 2):
                              t0b + (half + 1) * (S // 2)],
                    po[:],
                )

            def outproj_half(b, half, aos, rep):
                for ec in range(KC):
                    outproj_ec(b, half, ec, aos, rep)

            for rep in range(repeats):
                tiles = alloc_qkv(0)
                for t in (0, 1):
                    xth = xt00 if (t == 0 and rep == 0) else qkv_dma(0, t, rep)
                    for gi in range(3):
                        qkv_group(0, t, tiles, xth, gi, rep)
                vnat(0, tiles, rep, 0, 8)
                pro_fill = []
                for t in (2, 3):
                    xth = qkv_dma(0, t, rep)
                    for gi in range(3):
                        pro_fill.append(
                            (lambda t=t, xth=xth, gi=gi, tl=tiles:
                             qkv_group(0, t, tl, xth, gi, rep))
                        )
                pro_fill.append(
                    (lambda tl=tiles: vnat(0, tl, rep, 8, S // 128))
                )
                prev = None
                for b in range(B):
                    nxt = b + 1 if b + 1 < B else None
                    tiles_next = alloc_qkv(nxt) if nxt is not None else None
                    aos = aop.tile([128, S], f32r, tag="ao", name=f"ao{rep}_{b}")
                    for qb in range(NQB):
                        att = [
                            atp.tile([128, 16 * QB], bf16, tag=f"att{h}",
                                     name=f"att{rep}_{b}_{qb}_{h}")
                            for h in range(2)
                        ]
                        fill = []
                        if b == 0 and qb == 0:
                            fill.extend(pro_fill)
                        if nxt is not None:
                            tls = [0] if qb == 0 else [1, 2, 3]
                            for t in tls:
                                xth = qkv_dma(nxt, t, rep)
                                for gi in range(3):
                                    fill.append(
                                        (lambda t=t, xth=xth, gi=gi:
                                         qkv_group(nxt, t, tiles_next,
                                                   xth, gi, rep))
                                    )
                        if qb == 0 and prev is not None:
                            pb, paos = prev
                            for ec in range(KC):
                                fill.append(
                                    (lambda ec=ec, pb=pb, paos=paos:
                                     outproj_ec(pb, 1, ec, paos, rep,
                                                eng="alt"))
                                )
                            prev = None
                        if qb == 1:
                            for ec in range(KC):
                                fill.append(
                                    (lambda ec=ec: outproj_ec(b, 0, ec,
                                                              aos, rep,
                                                              eng="alt"))
                                )
                        scores(b, qb, tiles, att, rep, fill)
                        for qbb in range(QB // 512):
                            for h in range(2):
                                attv_qbb(b, qb, qbb, h, att, aos, rep)
                    if nxt is not None:
                        vnat(nxt, tiles_next, rep)
                    prev = (b, aos)
                    tiles = tiles_next
                pb, paos = prev
                outproj_half(pb, 1, paos, rep)
    nc.compile()
    return nc


_CACHE = {}


def _get_nc(repeats=1):
    if repeats not in _CACHE:
        _CACHE[repeats] = build(repeats)
    return _CACHE[repeats]


def make_in_maps(x, W_qkv, b_qkv, W_out, b_out):
    x = np.asarray(x, dtype=np.float32)
    W_qkv = np.asarray(W_qkv, dtype=np.float32)
    b_qkv = np.asarray(b_qkv, dtype=np.float32)
    W_out = np.asarray(W_out, dtype=np.float32)
    xT = np.ascontiguousarray(x.reshape(TOK, E).T).astype(ml_dtypes.bfloat16)
    trim = np.ascontiguousarray(
        np.triu(np.ones((128, 128), dtype=np.float32))
    ).astype(ml_dtypes.bfloat16)
    in_maps = []
    for c in range(8):
        cs = slice(c * 128, (c + 1) * 128)
        in_maps.append({
            "xT": xT,
            "wq": np.ascontiguousarray(
                W_qkv[:, c * 128:(c + 1) * 128]).astype(ml_dtypes.bfloat16),
            "wk": np.ascontiguousarray(
                W_qkv[:, E + c * 128:E + (c + 1) * 128]).astype(
                    ml_dtypes.bfloat16),
            "wv": np.ascontiguousarray(
                W_qkv[:, 2 * E + c * 128:2 * E + (c + 1) * 128]).astype(
                    ml_dtypes.bfloat16),
            "wo": np.ascontiguousarray(W_out[cs, :]),
            "bq": np.ascontiguousarray(b_qkv[c * 128:(c + 1) * 128, None]),
            "bk": np.ascontiguousarray(b_qkv[E + c * 128:E + (c + 1) * 128, None]),
            "bv": np.ascontiguousarray(
                b_qkv[2 * E + c * 128:2 * E + (c + 1) * 128, None]),
            "tri": trim,
            "idd": np.eye(128, dtype=np.float32).astype(ml_dtypes.bfloat16),
        })
    return in_maps


def gather(results, b_out):
    total = np.zeros((E, TOK), dtype=np.float64)
    for c in range(8):
        total += results[c]["outp"].astype(np.float64)
    out = total.T.astype(np.float32) + np.asarray(b_out, dtype=np.float32)
    return np.ascontiguousarray(out.reshape(B, S, E)).astype(np.float32)


def kernel(x, W_qkv, b_qkv, W_out, b_out):
    nc = _get_nc(1)
    in_maps = make_in_maps(x, W_qkv, b_qkv, W_out, b_out)
    res = bass_utils.run_bass_kernel_spmd(nc, in_maps, core_ids=list(range(8)))
    return gather(res.results, b_out)


# revision 13
# speedup vs baseline: 1.3358x; 1.0973x over previous
"""Phase-0 + per-qb att tiles (kills exp-vs-AV WAR serialization)."""
import numpy as np
import ml_dtypes

import concourse.bacc as bacc
import concourse.bass as bass
import concourse.mybir as mybir
import concourse.tile as tile
from concourse import bass_utils

B, S, E, H = 4, 2048, 1024, 16
D = E // H
TOK = B * S
KC = E // 128
TB = 512
QB = 1024
NB = S // TB
NQB = S // QB

f32 = mybir.dt.float32
f32r = mybir.dt.float32r
bf16 = mybir.dt.bfloat16
FT = mybir.ActivationFunctionType


def splits(lo, hi, step=512):
    out = []
    p = lo
    while p < hi:
        q = min((p // step + 1) * step, hi)
        out.append((p, q))
        p = q
    return out


def build(repeats: int = 1):
    nc = bacc.Bacc("TRN2", target_bir_lowering=False, debug=False, num_devices=8)
    xT = nc.dram_tensor("xT", [E, TOK], bf16, kind="ExternalInput")
    wq = nc.dram_tensor("wq", [E, 128], bf16, kind="ExternalInput")
    wk = nc.dram_tensor("wk", [E, 128], bf16, kind="ExternalInput")
    wv = nc.dram_tensor("wv", [E, 128], bf16, kind="ExternalInput")
    wo = nc.dram_tensor("wo", [128, E], f32r, kind="ExternalInput")
    bq = nc.dram_tensor("bq", [128, 1], f32, kind="ExternalInput")
    bk = nc.dram_tensor("bk", [128, 1], f32, kind="ExternalInput")
    bv = nc.dram_tensor("bv", [128, 1], f32, kind="ExternalInput")
    tri = nc.dram_tensor("tri", [128, 128], bf16, kind="ExternalInput")
    idd = nc.dram_tensor("idd", [128, 128], bf16, kind="ExternalInput")
    outp = nc.dram_tensor("outp", [E, TOK], bf16, kind="ExternalOutput")

    with tile.TileContext(nc) as tc:
        with (
            tc.tile_pool(name="wp", bufs=1) as wp,
            tc.tile_pool(name="xp", bufs=2) as xp,
            tc.tile_pool(name="qk", bufs=2) as qk,
            tc.tile_pool(name="vn", bufs=1) as vnp,
            tc.tile_pool(name="at", bufs=1) as atp,
            tc.tile_pool(name="ao", bufs=2) as aop,
            tc.tile_pool(name="ms", bufs=2) as ms,
            tc.tile_pool(name="op", bufs=3) as op,
            tc.tile_pool(name="psA", bufs=2, space="PSUM") as psA,
            tc.tile_pool(name="psS", bufs=1, space="PSUM") as psS,
            tc.tile_pool(name="psO", bufs=1, space="PSUM") as psO,
        ):
            xt00 = []
            for hf in range(2):
                x1 = xp.tile([128, KC * TB // 2], bf16, tag=f"xt{hf}",
                             name=f"xt_pre0_{hf}")
                nc.sync.dma_start(
                    x1[:].rearrange("p (c m) -> p c m", c=KC // 2),
                    xT.ap()[hf * (E // 2):(hf + 1) * (E // 2), 0:TB].rearrange(
                        "(c p) m -> p c m", p=128),
                )
                xt00.append(x1)
            wq_sb = wp.tile([128, E], bf16)
            wk_sb = wp.tile([128, E], bf16)
            wv_sb = wp.tile([128, E], bf16)
            wo_sb = wp.tile([128, E], f32r)
            bq_sb = wp.tile([128, 1], f32)
            bk_sb = wp.tile([128, 1], f32)
            bv_sb = wp.tile([128, 1], f32)
            for hf in range(2):
                nc.sync.dma_start(
                    wq_sb[:, hf * (E // 2):(hf + 1) * (E // 2)].rearrange(
                        "p (c m) -> p c m", c=KC // 2),
                    wq.ap()[hf * (E // 2):(hf + 1) * (E // 2), :].rearrange(
                        "(c p) m -> p c m", p=128),
                )
            nc.sync.dma_start(bq_sb[:], bq.ap())
            for wsb_, wdr_ in ((wk_sb, wk), (wv_sb, wv)):
                nc.sync.dma_start(
                    wsb_[:].rearrange("p (c m) -> p c m", c=KC),
                    wdr_.ap().rearrange("(c p) m -> p c m", p=128),
                )
            nc.sync.dma_start(wo_sb[:], wo.ap())
            nc.sync.dma_start(bk_sb[:], bk.ap())
            nc.sync.dma_start(bv_sb[:], bv.ap())
            tri_sb = wp.tile([128, 128], bf16)
            nc.sync.dma_start(tri_sb[:], tri.ap())
            id_sb = wp.tile([128, 128], bf16)
            nc.sync.dma_start(id_sb[:], idd.ap())
            warm = wp.tile([1, 1], f32)
            nc.vector.memset(warm[:], 0.0)
            nc.scalar.activation(warm[:], warm[:], FT.Exp, scale=1.0)
            vns = []
            for i in range(S // 128):
                vn = vnp.tile([128, 130], bf16, tag=f"vn{i}", name=f"vn{i}")
                nc.vector.memset(vn[:, 64:65], 1.0)
                nc.vector.memset(vn[:, 129:130], 1.0)
                vns.append(vn)

            def alloc_qkv(b):
                return (
                    qk.tile([128, S], f32r, tag="qT", name=f"qT{b}"),
                    qk.tile([128, S], f32r, tag="kT", name=f"kT{b}"),
                    qk.tile([128, S], bf16, tag="vT", name=f"vT{b}"),
                )

            def qkv_dma(b, t, rep):
                tok0 = b * S + t * TB
                xth = []
                for hf in range(2):
                    x1 = xp.tile([128, KC * TB // 2], bf16, tag=f"xt{hf}",
                                 name=f"xt{rep}_{b}_{t}_{hf}")
                    nc.sync.dma_start(
                        x1[:].rearrange("p (c m) -> p c m", c=KC // 2),
                        xT.ap()[hf * (E // 2):(hf + 1) * (E // 2),
                                tok0:tok0 + TB].rearrange(
                            "(c p) m -> p c m", p=128),
                    )
                    xth.append(x1)
                return xth

            def qkv_group(b, t, tiles, xth, gi, rep):
                qT, kT, vT = tiles
                wsb, bsb, dst = (
                    (wq_sb, bq_sb, qT), (wk_sb, bk_sb, kT),
                    (wv_sb, bv_sb, vT),
                )[gi]
                ps = psA.tile([128, TB], f32, tag="mm512",
                              name=f"psqkv{rep}_{b}_{t}_{gi}")
                for kc in range(KC):
                    xsrc = xth[kc // (KC // 2)]
                    nc.tensor.matmul(
                        ps[:],
                        wsb[:, kc * 128:(kc + 1) * 128],
                        xsrc[:, (kc % (KC // 2)) * TB:
                             (kc % (KC // 2) + 1) * TB],
                        start=(kc == 0), stop=(kc == KC - 1),
                    )
                nc.vector.tensor_scalar_add(
                    dst[:, t * TB:(t + 1) * TB], ps[:], bsb[:]
                )

            def vnat(b, tiles, rep, lo=0, hi=S // 128):
                vT = tiles[2]
                for i in range(lo, hi):
                    vn = vns[i]
                    pst = psA.tile([128, 128], bf16, tag="mm512",
                                   name=f"pst{rep}_{b}_{i}")
                    nc.tensor.transpose(
                        pst[:], vT[:, i * 128:(i + 1) * 128], id_sb[:]
                    )
                    dst = vn[:, 0:64]
                    dst3 = bass.AP(dst.tensor, dst.offset,
                                   [dst.ap[0], [65, 2], [1, 64]])
                    src = pst[:, 0:64]
                    src3 = bass.AP(src.tensor, src.offset,
                                   [src.ap[0], [64, 2], [1, 64]])
                    nc.vector.tensor_copy(dst3, src3)

            def scores(b, qb, tiles, att, rep, fill=()):
                qT, kT, vT = tiles
                q0 = qb * QB
                nkc = (q0 + QB) // 128
                fill = list(fill)
                nf = len(fill)
                fired = 0
                pss = {}
                for kc in range(nkc):
                    kst = kc * 128
                    r0 = max(0, kst - q0)
                    for h in range(2):
                        ps_s = psS.tile([128, QB], f32, tag=f"s{h}",
                                        name=f"pss{rep}_{b}_{qb}_{kc}_{h}")
                        hs = slice(h * 64, (h + 1) * 64)
                        for (p0, p1) in splits(r0, QB):
                            nc.tensor.matmul(
                                ps_s[:, p0:p1],
                                kT[hs, kst:kst + 128],
                                qT[hs, q0 + p0:q0 + p1],
                                start=True, stop=True,
                                tile_position=(h * 64, 0),
                            )
                        pss[(kc, h)] = ps_s
                    for h in range(2):
                        ps_s = pss[(kc, h)]
                        nc.scalar.activation(
                            att[h][:, kc * QB + r0:(kc + 1) * QB],
                            ps_s[:, r0:QB],
                            FT.Exp, scale=0.125,
                        )
                        if kst >= q0:
                            blk = att[h][:, kc * QB + r0:kc * QB + r0 + 128]
                            nc.vector.tensor_tensor(
                                blk, blk, tri_sb[:],
                                op=mybir.AluOpType.mult,
                            )
                    want = (kc + 1) * nf // nkc
                    while fired < want:
                        fill[fired]()
                        fired += 1
                while fired < nf:
                    fill[fired]()
                    fired += 1

            def attv_qbb(b, qb, qbb, h, att, aos, rep):
                q0 = qb * QB
                qa0 = q0 + qbb * 512
                nkc_q = (qa0 + 512) // 128
                ps_o = psO.tile([65, 512], f32, tag=f"o{h}",
                                name=f"pso{rep}_{b}_{qb}_{qbb}_{h}")
                for kc in range(nkc_q):
                    kst = kc * 128
                    lo = max(qa0, kst) - qa0
                    vn = vns[kc]
                    nc.tensor.matmul(
                        ps_o[:, lo:512],
                        vn[:, h * 65:(h + 1) * 65],
                        att[h][:, kc * QB + qbb * 512 + lo:
                               kc * QB + (qbb + 1) * 512],
                        start=(kc == 0), stop=(kc == nkc_q - 1),
                    )
                rec = ms.tile([1, 512], f32, tag=f"rec{h}",
                              name=f"rec{rep}_{b}_{qb}_{qbb}_{h}")
                nc.vector.reciprocal(rec[:], ps_o[64:65, :])
                bc = ms.tile([64, 512], f32, tag=f"bc{h}",
                             name=f"bc{rep}_{b}_{qb}_{qbb}_{h}")
                nc.gpsimd.partition_broadcast(bc[:], rec[:])
                nc.vector.tensor_tensor(
                    aos[h * 64:(h + 1) * 64, qa0:qa0 + 512],
                    ps_o[0:64, :], bc[:],
                    op=mybir.AluOpType.mult,
                )

            def outproj_ec(b, half, ec, aos, rep, eng="alt"):
                t0b = b * S
                po = op.tile([128, S // 2], bf16, tag="po",
                             name=f"po{rep}_{b}_{ec}_{half}")
                for tt in range(NB // 2):
                    t = half * (NB // 2) + tt
                    ps_p = psA.tile([128, TB], f32, tag="mm512",
                                    name=f"psp{rep}_{b}_{ec}_{t}")
                    nc.tensor.matmul(
                        ps_p[:],
                        wo_sb[:, ec * 128:(ec + 1) * 128],
                        aos[:, t * TB:(t + 1) * TB],
                        start=True, stop=True,
                    )
                    if eng == "alt" and (ec * 2 + tt) % 4 == 3:
                        nc.scalar.copy(
                            po[:, tt * TB:(tt + 1) * TB], ps_p[:]
                        )
                    else:
                        nc.vector.tensor_copy(
                            po[:, tt * TB:(tt + 1) * TB], ps_p[:]
                        )
                nc.sync.dma_start(
                    outp.ap()[ec * 128:(ec + 1) * 128,
                              t0b + half * (S // 2):
                              t0b + (half + 1) * (S // 2)],
                    po[:],
                )

            def outproj_half(b, half, aos, rep):
                for ec in range(KC):
                    outproj_ec(b, half, ec, aos, rep)

            for rep in range(repeats):
                tiles = alloc_qkv(0)
                for t in (0, 1):
                    xth = xt00 if (t == 0 and rep == 0) else qkv_dma(0, t, rep)
                    for gi in range(3):
                        qkv_group(0, t, tiles, xth, gi, rep)
                vnat(0, tiles, rep, 0, 8)
                pro_fill = []
                for t in (2, 3):
                    xth = qkv_dma(0, t, rep)
                    for gi in range(3):
                        pro_fill.append(
                            (lambda t=t, xth=xth, gi=gi, tl=tiles:
                             qkv_group(0, t, tl, xth, gi, rep))
                        )
                pro_fill.append(
                    (lambda tl=tiles: vnat(0, tl, rep, 8, S // 128))
                )
                prev = None
                for b in range(B):
                    nxt = b + 1 if b + 1 < B else None
                    tiles_next = alloc_qkv(nxt) if nxt is not None else None
                    aos = aop.tile([128, S], f32r, tag="ao", name=f"ao{rep}_{b}")
                    for qb in range(NQB):
                        natt = 8 if qb == 0 else 16
                        att = [
                            atp.tile([128, natt * QB], bf16,
                                     tag=f"att{qb}_{h}",
                                     name=f"att{rep}_{b}_{qb}_{h}")
                            for h in range(2)
                        ]
                        fill = []
                        if b == 0 and qb == 0:
                            fill.extend(pro_fill)
                        if nxt is not None:
                            tls = [0] if qb == 0 else [1, 2, 3]
                            for t in tls:
                                xth = qkv_dma(nxt, t, rep)
                                for gi in range(3):
                                    fill.append(
                                        (lambda t=t, xth=xth, gi=gi:
                                         qkv_group(nxt, t, tiles_next,
                                                   xth, gi, rep))
                                    )
                        if qb == 0 and prev is not None:
                            pb, paos = prev
                            for ec in range(KC):
                                fill.append(
                                    (lambda ec=ec, pb=pb, paos=paos:
                                     outproj_ec(pb, 1, ec, paos, rep,
                                                eng="alt"))
                                )
                            prev = None
                        if qb == 1:
                            for ec in range(KC):
                                fill.append(
                                    (lambda ec=ec: outproj_ec(b, 0, ec,
                                                              aos, rep,
                                                              eng="alt"))
                                )
                        scores(b, qb, tiles, att, rep, fill)
                        for qbb in range(QB // 512):
                            for h in range(2):
                                attv_qbb(b, qb, qbb, h, att, aos, rep)
                    if nxt is not None:
                        vnat(nxt, tiles_next, rep)
                    prev = (b, aos)
                    tiles = tiles_next
                pb, paos = prev
                outproj_half(pb, 1, paos, rep)
    nc.compile()
    return nc


_CACHE = {}


def _get_nc(repeats=1):
    if repeats not in _CACHE:
        _CACHE[repeats] = build(repeats)
    return _CACHE[repeats]


def make_in_maps(x, W_qkv, b_qkv, W_out, b_out):
    x = np.asarray(x, dtype=np.float32)
    W_qkv = np.asarray(W_qkv, dtype=np.float32)
    b_qkv = np.asarray(b_qkv, dtype=np.float32)
    W_out = np.asarray(W_out, dtype=np.float32)
    xT = np.ascontiguousarray(x.reshape(TOK, E).T).astype(ml_dtypes.bfloat16)
    trim = np.ascontiguousarray(
        np.triu(np.ones((128, 128), dtype=np.float32))
    ).astype(ml_dtypes.bfloat16)
    in_maps = []
    for c in range(8):
        cs = slice(c * 128, (c + 1) * 128)
        in_maps.append({
            "xT": xT,
            "wq": np.ascontiguousarray(
                W_qkv[:, c * 128:(c + 1) * 128]).astype(ml_dtypes.bfloat16),
            "wk": np.ascontiguousarray(
                W_qkv[:, E + c * 128:E + (c + 1) * 128]).astype(
                    ml_dtypes.bfloat16),
            "wv": np.ascontiguousarray(
                W_qkv[:, 2 * E + c * 128:2 * E + (c + 1) * 128]).astype(
                    ml_dtypes.bfloat16),
            "wo": np.ascontiguousarray(W_out[cs, :]),
            "bq": np.ascontiguousarray(b_qkv[c * 128:(c + 1) * 128, None]),
            "bk": np.ascontiguousarray(b_qkv[E + c * 128:E + (c + 1) * 128, None]),
            "bv": np.ascontiguousarray(
                b_qkv[2 * E + c * 128:2 * E + (c + 1) * 128, None]),
            "tri": trim,
            "idd": np.eye(128, dtype=np.float32).astype(ml_dtypes.bfloat16),
        })
    return in_maps


def gather(results, b_out):
    total = np.zeros((E, TOK), dtype=np.float64)
    for c in range(8):
        total += results[c]["outp"].astype(np.float64)
    out = total.T.astype(np.float32) + np.asarray(b_out, dtype=np.float32)
    return np.ascontiguousarray(out.reshape(B, S, E)).astype(np.float32)


def kernel(x, W_qkv, b_qkv, W_out, b_out):
    nc = _get_nc(1)
    in_maps = make_in_maps(x, W_qkv, b_qkv, W_out, b_out)
    res = bass_utils.run_bass_kernel_spmd(nc, in_maps, core_ids=list(range(8)))
    return gather(res.results, b_out)
